# revision 28
# baseline (speedup 1.0000x reference)
"""Bahdanau (additive) attention TRN2 Bass kernel (v10).

reference:
    proj_in = einsum("bse,ea->bsa", inputs, W_in)      # [B,S,A]
    proj_q  = (query @ W_q)[:, None, :]                # [B,1,A]
    scores  = einsum("bsa,a->bs", tanh(proj_in+proj_q), w_att)
    weights = softmax(scores, axis=1)
    context = einsum("bs,bsa->ba", weights, proj_in)   # [B,A]

B,S,E,Q,A = 32,2048,1024,1024,512.

Sharding: data-parallel over batch. 8 cores x 4 batches each; weights
replicated. No collectives; host scatters inputs / gathers outputs.

Measured mechanics this kernel is built around (v6..v9c traces):
  - DMA descriptor generation is ~260ns/descriptor/queue at the head:
    all DRAM operands are host-laid so every partition line is one
    long contiguous descriptor (x: 16KB quads / 8KB b0-pairs; w_in:
    2KB at-slices; projq 64B; watt 8B).
  - Every hardware-DGE DMA's completion semaphore lands ~2us (early)
    to ~10us (late kernel) after its data; the end-of-kernel barrier
    waits on ALL of them, so the final store eats the full lag. The
    last x/wbc/store DMAs are therefore issued as early as possible.
  - The in-order PE queue must never hold an instruction whose wait
    depends on ACT/DVE progress while main MMs remain (denominator
    matmuls go after the carrier's main MMs; the final scores bank is
    memset early).
  - ACT op overhead ~350cyc -> tanh batched [P,S] per at; ACT reduces
    use the per-partition scale operand to fold the softmax normalize.
  - HAM: ~3.4us of sustained PE activity re-clocks 1.2->2.4GHz; DMA-
    independent dummy MMs warm it during the load ramp.
  - proj_q is computed on the host (trivial 32x512 matmul).

Structure:
  - per batch: 2 quad x-DMAs (b0: 4 pair-DMAs); at-groups ec-outer
    sc-inner accumulating 4 PSUM banks; DVE casts PSUM->projTall bf16
    (sole PSUM reader); one batched tanh per at (bias fused).
  - deferred epilogue for batch b runs in carrier b+1: scores via
    col-tiled stripes into ONE PSUM bank (tile_position), whole-bank
    exp + esum, DRAM-bounce broadcast of the weights, PE mask-dot
    denominator, DVE multiply + ACT scale-reduce ctx, scalar-ring
    store.
  - b2's epilogue interleaves into b3's at2/at3 windows (fold-halved
    reduces) so the final epilogue's exp isn't queued behind it.
  - final epilogue: sc-outer last at-group, per-sc scores/exp/PE-
    broadcast pipeline, ctx on 4 scratch tiles, single-packet store.
"""

import sys

sys.path.insert(0, "/opt/trn_rl_repo")

import ml_dtypes
import numpy as np

import concourse.bass as bass
import concourse.tile as tile
from concourse import bacc, bass_utils, mybir

B, S, E, Q, A = 32, 2048, 1024, 1024, 512
NCORES = 8
BPC = B // NCORES  # batches per core
P = 128
EC = E // P  # 8 e-chunks
AT = A // P  # 4 a-tiles
SF = 512  # matmul moving free dim
SC = S // SF  # 4 s-chunks
NQ = EC // 4  # quad tiles per batch (2)

BF = mybir.dt.bfloat16
F32 = mybir.dt.float32
TANH = mybir.ActivationFunctionType.Tanh
EXP = mybir.ActivationFunctionType.Exp
COPY = mybir.ActivationFunctionType.Copy
MULT = mybir.AluOpType.mult
ADD = mybir.AluOpType.add


def build():
    nc = bacc.Bacc("TRN2", target_bir_lowering=False, debug=False)

    xT = nc.dram_tensor("xT", [BPC, NQ, P, 4 * S], BF, kind="ExternalInput")
    # w_in host-laid [p, at, ec, j]: per-at slices are 2KB/partition
    w_in = nc.dram_tensor("w_in", [P, AT, EC, P], BF, kind="ExternalInput")
    # w_att host-laid [p, at]: 8B/partition lines
    w_att = nc.dram_tensor("w_att", [P, AT], BF, kind="ExternalInput")
    projq_in = nc.dram_tensor("projq", [P, AT, BPC], F32, kind="ExternalInput")
    out = nc.dram_tensor("out", [BPC, A], F32, kind="ExternalOutput")

    with tile.TileContext(nc) as tc:
        with (
            tc.tile_pool(name="const", bufs=1) as const,
            tc.tile_pool(name="xtp", bufs=2) as xtp,
            tc.tile_pool(name="ttp", bufs=2) as ttp,
            tc.tile_pool(name="small", bufs=3) as small,
            tc.tile_pool(name="mm_ps", bufs=7, space="PSUM") as mm_ps,
            tc.tile_pool(name="sc_ps", bufs=1, space="PSUM") as sc_ps,
            tc.tile_pool(name="dram", bufs=2, space="DRAM") as dram,
        ):
            # ---- tiny constants + HAM warmup (no DMA dependency) -----
            ones2 = const.tile([P, P], BF)
            nc.vector.memset(ones2, 1.0)
            dummy = const.tile([P, SF], BF)
            nc.vector.memset(dummy, 0.0)
            warm_ps = mm_ps.tile([P, SF], F32, name="mm_acc")
            NWARM = 14
            for i in range(NWARM):
                nc.tensor.matmul(
                    warm_ps, ones2, dummy, start=(i == 0), stop=(i == NWARM - 1)
                )
            # tiny reader so the warmup bank has a tracked release (DVE:
            # on the Scalar queue it delays the weight DMA issues)
            warm_scr = small.tile([1, 1], F32, name="warm_scr")
            nc.vector.tensor_copy(warm_scr, warm_ps[:1, :1])

            ones_f = const.tile([1, P], F32)
            nc.vector.memset(ones_f, 1.0)
            mask_f = const.tile([P, 1], F32)
            nc.vector.memset(mask_f, 0.0)
            for sc in range(SC):
                nc.vector.memset(mask_f[32 * sc : 32 * sc + 1, :], 1.0)

            # ---- weights on the scalar ring: at0's slice first so the
            # first MM group is gated on 256KB, not 1MB.
            w_sb = const.tile([P, AT, EC, P], BF)
            nc.scalar.dma_start(w_sb[:, :1], w_in.ap()[:, :1])
            nc.scalar.dma_start(w_sb[:, 1:], w_in.ap()[:, 1:])
            projq = const.tile([P, AT, BPC], F32)
            nc.scalar.dma_start(projq, projq_in.ap())
            watt_sb = const.tile([P, AT], BF)
            nc.scalar.dma_start(watt_sb, w_att.ap())

            # ---- epilogue pieces -------------------------------------
            def emit_scores(pts, sps=None, final=False):
                """Col-tiled scores: ONE PSUM bank, 4 stripes at partitions
                {0,32,64,96} via tile_position, start=False onto a zeroed
                bank. Deferred (final=False): at-outer stripe order, one
                whole-bank exp + esum. Final: sc-outer so each stripe
                chain closes as its tanh lands; per-stripe exp [1,SF] +
                per-stripe esum let the PE broadcast start immediately."""
                if sps is None:
                    sps = sc_ps.tile([P, SF], F32, name="sps")
                    nc.vector.memset(sps, 0.0)
                if not final:
                    for at in range(AT):
                        for sc in range(SC):
                            nc.tensor.matmul(
                                sps[32 * sc : 32 * sc + 1, :],
                                watt_sb[:, at : at + 1],
                                pts[at][:, sc * SF : (sc + 1) * SF],
                                start=False,
                                stop=(at == AT - 1),
                                skip_group_check=True,
                                tile_position=(0, 32 * sc),
                            )
                    exp_sb = small.tile([P, SF], BF, name="exp_sb")
                    esum = small.tile([P, 1], F32, name="esum")
                    nc.scalar.activation(exp_sb, sps, EXP, accum_out=esum)
                    return exp_sb, esum
                exp_sb = small.tile([P, SF], BF, name="exp_sb")
                esum = small.tile([P, 1], F32, name="esum")
                # per-stripe exp writes only rows {0,32,64,96}; the
                # mask-dot reads all 128 rows
                nc.vector.memset(esum, 0.0)
                for sc in range(SC):
                    for at in range(AT):
                        nc.tensor.matmul(
                            sps[32 * sc : 32 * sc + 1, :],
                            watt_sb[:, at : at + 1],
                            pts[at][:, sc * SF : (sc + 1) * SF],
                            start=False,
                            stop=(at == AT - 1),
                            skip_group_check=True,
                            tile_position=(0, 32 * sc),
                        )
                    nc.scalar.activation(
                        exp_sb[32 * sc : 32 * sc + 1, :],
                        sps[32 * sc : 32 * sc + 1, :],
                        EXP,
                        accum_out=esum[32 * sc : 32 * sc + 1, :],
                    )
                return exp_sb, esum

            def emit_wbc_dma(exp_sb):
                # gather the 4 stripe rows into DRAM, broadcast back to
                # all 128 partitions (stride-0 read); scalar ring
                exp_dram = dram.tile([1, S], BF, name="exp_dram")
                nc.scalar.dma_start(
                    bass.AP(
                        tensor=exp_dram.tensor,
                        offset=exp_dram.offset,
                        ap=[[SF, SC], [1, SF]],
                    ),
                    exp_sb[0 : 32 * SC - 31 : 32, :],
                )
                wbc = ttp.tile([P, S], BF, name="wbc")
                nc.scalar.dma_start(
                    wbc,
                    bass.AP(
                        tensor=exp_dram.tensor,
                        offset=exp_dram.offset,
                        ap=[[0, P], [1, S]],
                    ),
                )
                return wbc

            def emit_rcp(esum):
                """softmax denominator: tot = mask . esum (stripe rows),
                broadcast with a K=1 ones matmul, reciprocal on [128,1].
                Tiny PE matmuls; call only at points where ACT's exp has
                long completed (the in-order PE queue would stall)."""
                tot_ps = mm_ps.tile([P, SF], F32, name="mm_acc")
                nc.tensor.matmul(tot_ps[:1, :1], mask_f, esum, start=True, stop=True)
                tot_sb = small.tile([1, 1], F32, name="tot_sb")
                nc.scalar.copy(tot_sb, tot_ps[:1, :1])
                totbc_ps = mm_ps.tile([P, SF], F32, name="mm_acc")
                nc.tensor.matmul(totbc_ps[:, :1], ones_f, tot_sb, start=True, stop=True)
                totbc = small.tile([P, 1], F32, name="totbc")
                nc.vector.tensor_copy(totbc, totbc_ps[:, :1])
                rcp = small.tile([P, 1], F32, name="rcp")
                nc.vector.reciprocal(rcp, totbc)
                return rcp

            def ctx_mult(proj, wbc, scr, i):
                nc.vector.tensor_tensor(
                    out=scr, in0=proj[:, i * S : (i + 1) * S], in1=wbc, op=MULT
                )

            def ctx_reduce(scr, rcp, c, i, fold=False):
                if fold:
                    nc.vector.tensor_tensor(
                        out=scr[:, : S // 2], in0=scr[:, : S // 2],
                        in1=scr[:, S // 2 :], op=ADD,
                    )
                    nc.scalar.activation(
                        scr[:, : S // 2], scr[:, : S // 2], COPY, scale=rcp,
                        accum_out=c[:, i : i + 1],
                    )
                else:
                    nc.scalar.activation(
                        scr, scr, COPY, scale=rcp, accum_out=c[:, i : i + 1]
                    )

            def emit_store(c, bidx, single=False):
                nc.scalar.dma_start(
                    bass.AP(tensor=out, offset=bidx * A, ap=[[1, P], [P, AT]]),
                    c,
                    single_packet=single,
                )

            # ---- main batch loop -------------------------------------
            prev = None  # (batch_idx, t tiles, projTall)
            ep = {}  # in-flight deferred epilogue state
            for b in range(BPC):
                last = b == BPC - 1
                xquads = []
                for h in range(NQ):
                    xq = xtp.tile([P, 4, S], BF, name=f"xq{h}")
                    nparts = 2 if b == 0 else 1
                    step = 4 // nparts
                    for j in range(nparts):
                        nc.sync.dma_start(
                            xq[:, j * step : (j + 1) * step, :],
                            bass.AP(
                                tensor=xT,
                                offset=((b * NQ + h) * P) * (4 * S) + j * step * S,
                                ap=[[4 * S, P], [1, step * S]],
                            ),
                        )
                    xquads.append(xq)

                ts_ = []
                projTall = ttp.tile([P, AT * S], BF, name="projTall", bufs=3)
                for at in range(AT):
                    t_sb = ttp.tile([P, S], BF, name=f"t{at}")
                    pss = [mm_ps.tile([P, SF], F32, name="mm_acc") for _ in range(SC)]
                    sl_at = slice(at * S, (at + 1) * S)
                    if last and at == AT - 1:
                        # sc-OUTER: each s-chunk closes early; tanh reads
                        # its PSUM bank (bias fused) per chunk; b2's
                        # interleaved reduces fill the ACT slack
                        for sc in range(SC):
                            for ec in range(EC):
                                nc.tensor.matmul(
                                    pss[sc],
                                    w_sb[:, at, ec, :],
                                    xquads[ec // 4][
                                        :, ec % 4, sc * SF : (sc + 1) * SF
                                    ],
                                    start=(ec == 0),
                                    stop=(ec == EC - 1),
                                )
                            sl = slice(at * S + sc * SF, at * S + (sc + 1) * SF)
                            nc.scalar.activation(
                                t_sb[:, sc * SF : (sc + 1) * SF],
                                pss[sc],
                                TANH,
                                bias=projq[:, at, b : b + 1],
                            )
                            nc.vector.tensor_copy(projTall[:, sl], pss[sc])
                            if prev is not None:
                                # b2 epilogue: reduce sc (fold-halved) +
                                # late multiplies as scratch frees
                                ctx_reduce(
                                    ep["cscr"][sc % 2], ep["rcp"], ep["c"], sc,
                                    fold=True,
                                )
                                if sc + 2 < AT:
                                    ctx_mult(
                                        ep["proj"], ep["wbc"],
                                        ep["cscr"][sc % 2], sc + 2,
                                    )
                        if prev is not None:
                            emit_store(ep["c"], ep["b"])
                    else:
                        for ec in range(EC):
                            for sc in range(SC):
                                nc.tensor.matmul(
                                    pss[sc],
                                    w_sb[:, at, ec, :],
                                    xquads[ec // 4][
                                        :, ec % 4, sc * SF : (sc + 1) * SF
                                    ],
                                    start=(ec == 0),
                                    stop=(ec == EC - 1),
                                )
                        # single PSUM reader (DVE cast) gates PSUM release;
                        # tanh reads the SBUF copy in ONE whole-row ACT op
                        for sc in range(SC):
                            sl = slice(at * S + sc * SF, at * S + (sc + 1) * SF)
                            nc.vector.tensor_copy(projTall[:, sl], pss[sc])
                        nc.scalar.activation(
                            t_sb,
                            projTall[:, sl_at],
                            TANH,
                            bias=projq[:, at, b : b + 1],
                        )
                    ts_.append(t_sb)

                    if at == 1 and prev is not None:
                        # deferred epilogue part 1: scores / exp / broadcast
                        ep["b"], ep["ts"], ep["proj"] = prev
                        ep["exp"], ep["esum"] = emit_scores(ep["ts"])
                        ep["wbc"] = emit_wbc_dma(ep["exp"])
                        ep["cscr"] = [
                            ttp.tile([P, S], BF, name=f"cscr{i}", bufs=1)
                            for i in range(2)
                        ]
                        if last:
                            # pre-allocate + memset the FINAL scores bank
                            # (its memset must not queue behind epilogue
                            # DVE work and idle the PE)
                            ep["fsps"] = sc_ps.tile([P, SF], F32, name="sps")
                            nc.vector.memset(ep["fsps"], 0.0)

                    if at == 2 and last and prev is not None:
                        # b2 epilogue spread: denominator + first two
                        # multiplies during the at2 window
                        ep["rcp"] = emit_rcp(ep["esum"])
                        ep["c"] = small.tile([P, AT], F32, name="c")
                        ctx_mult(ep["proj"], ep["wbc"], ep["cscr"][0], 0)
                        ctx_mult(ep["proj"], ep["wbc"], ep["cscr"][1], 1)

                # non-last carriers: whole deferred ctx after the main MMs
                if prev is not None and not last:
                    ep["rcp"] = emit_rcp(ep["esum"])
                    ep["c"] = small.tile([P, AT], F32, name="c")
                    for i in range(AT):
                        scr = ep["cscr"][i % 2]
                        ctx_mult(ep["proj"], ep["wbc"], scr, i)
                        ctx_reduce(scr, ep["rcp"], ep["c"], i)
                    emit_store(ep["c"], ep["b"])

                prev = (b, ts_, projTall)

            # ---- final epilogue (latency-critical, PE idle afterwards) --
            pb, pts, pproj = prev
            exp_sb, esum = emit_scores(pts, sps=ep["fsps"], final=True)
            # PE K=1 ones-matmul broadcast of the exp stripe rows, per-sc
            # (each gated only on its stripe's exp)
            wbc = ttp.tile([P, S], BF, name="wbc")
            wpss = []
            for sc in range(SC):
                wps = mm_ps.tile([P, SF], F32, name="mm_acc")
                nc.tensor.matmul(
                    wps,
                    ones2[32 * sc : 32 * sc + 1, :],
                    exp_sb[32 * sc : 32 * sc + 1, :],
                    start=True,
                    stop=True,
                    tile_position=(32 * sc, 0),
                )
                wpss.append(wps)
            rcp = emit_rcp(esum)
            for sc in range(SC):
                dst = wbc[:, sc * SF : (sc + 1) * SF]
                if sc % 2 == 0:
                    nc.vector.tensor_copy(dst, wpss[sc])
                else:
                    nc.scalar.copy(dst, wpss[sc])

            # final ctx on 4 scratch tiles (no WAR serialization); first
            # multiply chunked so it starts on the first broadcast chunk
            cscrs = [ttp.tile([P, S], BF, name=f"fscr{i}", bufs=1) for i in range(AT)]
            c = small.tile([P, AT], F32, name="c")
            for sc in range(SC):
                sl = slice(sc * SF, (sc + 1) * SF)
                nc.vector.tensor_tensor(
                    out=cscrs[0][:, sl], in0=pproj[:, sl], in1=wbc[:, sl], op=MULT
                )
            for i in (1, 2, 3):
                ctx_mult(pproj, wbc, cscrs[i], i)
            ctx_reduce(cscrs[0], rcp, c, 0)
            ctx_reduce(cscrs[2], rcp, c, 2)
            ctx_reduce(cscrs[1], rcp, c, 1, fold=True)
            ctx_reduce(cscrs[3], rcp, c, 3, fold=True)
            emit_store(c, pb, single=True)

    nc.compile()
    return nc


_nc = None


def prep_in_maps(inputs, query, W_in, W_q, w_att):
    """Host-side shard + pre-layout: every DRAM operand is arranged so
    each partition's line is one long contiguous DMA descriptor."""
    bf = ml_dtypes.bfloat16
    x_bf = np.asarray(inputs).astype(bf)
    # [b, s, e] -> [b, e, s] -> chunks [b, c, p, s] -> [b, h, p, j, s]
    xT_bf = x_bf.transpose(0, 2, 1).reshape(B, NQ, 4, P, S).transpose(0, 1, 3, 2, 4)
    xT_bf = np.ascontiguousarray(xT_bf).reshape(B, NQ, P, 4 * S)
    # w_in[e, a] -> [p, at, ec, j] with e = ec*128+p, a = at*128+j
    w_in_pre = np.ascontiguousarray(
        np.asarray(W_in).astype(bf).reshape(EC, P, AT, P).transpose(1, 2, 0, 3)
    )
    # w_att[a] -> [p, at] with a = at*128+p
    w_att_pre = np.ascontiguousarray(
        np.asarray(w_att).astype(bf).reshape(AT, P).T
    )
    # proj_q on host, in bf16-rounded operands to match device numerics
    pq = (
        np.asarray(query).astype(bf).astype(np.float32)
        @ np.asarray(W_q).astype(bf).astype(np.float32)
    )  # [B, A] f32

    in_maps = []
    for c in range(NCORES):
        sl = slice(c * BPC, (c + 1) * BPC)
        # proj_q[b, a] -> [p, at, b] with a = at*128+p
        pq_pre = np.ascontiguousarray(
            pq[sl].reshape(BPC, AT, P).transpose(2, 1, 0).astype(np.float32)
        )
        in_maps.append(
            {
                "xT": xT_bf[sl],
                "w_in": w_in_pre,
                "w_att": w_att_pre,
                "projq": pq_pre,
            }
        )
    return in_maps


def kernel(inputs, query, W_in, W_q, w_att):
    global _nc
    if _nc is None:
        _nc = build()
    in_maps = prep_in_maps(inputs, query, W_in, W_q, w_att)
    res = bass_utils.run_bass_kernel_spmd(_nc, in_maps, core_ids=list(range(NCORES)))
    return np.concatenate([r["out"] for r in res.results], axis=0)


if __name__ == "__main__":
    rng = np.random.default_rng(0)
    ins = {
        "inputs": rng.standard_normal((B, S, E), dtype=np.float32),
        "query": rng.standard_normal((B, Q), dtype=np.float32),
        "W_in": (rng.standard_normal((E, A), dtype=np.float32) / np.sqrt(E)).astype(
            np.float32
        ),
        "W_q": (rng.standard_normal((Q, A), dtype=np.float32) / np.sqrt(Q)).astype(
            np.float32
        ),
        "w_att": (rng.standard_normal((A,), dtype=np.float32) / np.sqrt(A)).astype(
            np.float32
        ),
    }
    got = kernel(**ins)
    print("out shape", got.shape, got.dtype)


# revision 32
# speedup vs baseline: 1.0530x; 1.0530x over previous
"""Bahdanau (additive) attention TRN2 Bass kernel (v10).

reference:
    proj_in = einsum("bse,ea->bsa", inputs, W_in)      # [B,S,A]
    proj_q  = (query @ W_q)[:, None, :]                # [B,1,A]
    scores  = einsum("bsa,a->bs", tanh(proj_in+proj_q), w_att)
    weights = softmax(scores, axis=1)
    context = einsum("bs,bsa->ba", weights, proj_in)   # [B,A]

B,S,E,Q,A = 32,2048,1024,1024,512.

Sharding: data-parallel over batch. 8 cores x 4 batches each; weights
replicated. No collectives; host scatters inputs / gathers outputs.

Measured mechanics this kernel is built around (v6..v9c traces):
  - DMA descriptor generation is ~260ns/descriptor/queue at the head:
    all DRAM operands are host-laid so every partition line is one
    long contiguous descriptor (x: 16KB quads / 8KB b0-pairs; w_in:
    2KB at-slices; projq 64B; watt 8B).
  - Every hardware-DGE DMA's completion semaphore lands ~2us (early)
    to ~10us (late kernel) after its data; the end-of-kernel barrier
    waits on ALL of them, so the final store eats the full lag. The
    last x/wbc/store DMAs are therefore issued as early as possible.
  - The in-order PE queue must never hold an instruction whose wait
    depends on ACT/DVE progress while main MMs remain (denominator
    matmuls go after the carrier's main MMs; the final scores bank is
    memset early).
  - ACT op overhead ~350cyc -> tanh batched [P,S] per at; ACT reduces
    use the per-partition scale operand to fold the softmax normalize.
  - HAM: ~3.4us of sustained PE activity re-clocks 1.2->2.4GHz; DMA-
    independent dummy MMs warm it during the load ramp.
  - proj_q is computed on the host (trivial 32x512 matmul).

Structure:
  - per batch: 2 quad x-DMAs (b0: 4 pair-DMAs); at-groups ec-outer
    sc-inner accumulating 4 PSUM banks; DVE casts PSUM->projTall bf16
    (sole PSUM reader); one batched tanh per at (bias fused).
  - deferred epilogue for batch b runs in carrier b+1: scores via
    col-tiled stripes into ONE PSUM bank (tile_position), whole-bank
    exp + esum, DRAM-bounce broadcast of the weights, PE mask-dot
    denominator, DVE multiply + ACT scale-reduce ctx, scalar-ring
    store.
  - b2's epilogue interleaves into b3's at2/at3 windows (fold-halved
    reduces) so the final epilogue's exp isn't queued behind it.
  - final epilogue: sc-outer last at-group, per-sc scores/exp/PE-
    broadcast pipeline, ctx on 4 scratch tiles, single-packet store.
"""

import sys

sys.path.insert(0, "/opt/trn_rl_repo")

import ml_dtypes
import numpy as np

import concourse.bass as bass
import concourse.tile as tile
from concourse import bacc, bass_utils, mybir

B, S, E, Q, A = 32, 2048, 1024, 1024, 512
NCORES = 8
BPC = B // NCORES  # batches per core
P = 128
EC = E // P  # 8 e-chunks
AT = A // P  # 4 a-tiles
SF = 512  # matmul moving free dim
SC = S // SF  # 4 s-chunks
NQ = EC // 4  # quad tiles per batch (2)

BF = mybir.dt.bfloat16
F32 = mybir.dt.float32
TANH = mybir.ActivationFunctionType.Tanh
EXP = mybir.ActivationFunctionType.Exp
COPY = mybir.ActivationFunctionType.Copy
MULT = mybir.AluOpType.mult
ADD = mybir.AluOpType.add


def build():
    nc = bacc.Bacc("TRN2", target_bir_lowering=False, debug=False)

    xT = nc.dram_tensor("xT", [BPC, NQ, P, 4 * S], BF, kind="ExternalInput")
    # w_in host-laid [p, at, ec, j]: per-at slices are 2KB/partition
    w_in = nc.dram_tensor("w_in", [P, AT, EC, P], BF, kind="ExternalInput")
    # w_att host-laid [p, at]: 8B/partition lines
    w_att = nc.dram_tensor("w_att", [P, AT], BF, kind="ExternalInput")
    projq_in = nc.dram_tensor("projq", [P, AT, BPC], F32, kind="ExternalInput")
    out = nc.dram_tensor("out", [BPC, A], F32, kind="ExternalOutput")

    with tile.TileContext(nc) as tc:
        with (
            tc.tile_pool(name="const", bufs=1) as const,
            tc.tile_pool(name="xtp", bufs=2) as xtp,
            tc.tile_pool(name="ttp", bufs=2) as ttp,
            tc.tile_pool(name="small", bufs=3) as small,
            tc.tile_pool(name="mm_ps", bufs=7, space="PSUM") as mm_ps,
            tc.tile_pool(name="sc_ps", bufs=1, space="PSUM") as sc_ps,
            tc.tile_pool(name="dram", bufs=2, space="DRAM") as dram,
        ):
            # ---- tiny constants + HAM warmup (no DMA dependency) -----
            ones2 = const.tile([P, P], BF)
            nc.vector.memset(ones2, 1.0)
            dummy = const.tile([P, SF], BF)
            nc.vector.memset(dummy, 0.0)
            warm_ps = mm_ps.tile([P, SF], F32, name="mm_acc")
            NWARM = 14
            for i in range(NWARM):
                nc.tensor.matmul(
                    warm_ps, ones2, dummy, start=(i == 0), stop=(i == NWARM - 1)
                )
            # tiny reader so the warmup bank has a tracked release (DVE:
            # on the Scalar queue it delays the weight DMA issues)
            warm_scr = small.tile([1, 1], F32, name="warm_scr")
            nc.vector.tensor_copy(warm_scr, warm_ps[:1, :1])

            ones_f = const.tile([1, P], F32)
            nc.vector.memset(ones_f, 1.0)
            mask_f = const.tile([P, 1], F32)
            nc.vector.memset(mask_f, 0.0)
            for sc in range(SC):
                nc.vector.memset(mask_f[32 * sc : 32 * sc + 1, :], 1.0)

            # ---- weights on the scalar ring: at0's slice first so the
            # first MM group is gated on 256KB, not 1MB.
            w_sb = const.tile([P, AT, EC, P], BF)
            nc.scalar.dma_start(w_sb[:, :1], w_in.ap()[:, :1])
            nc.scalar.dma_start(w_sb[:, 1:], w_in.ap()[:, 1:])
            projq = const.tile([P, AT, BPC], F32)
            nc.scalar.dma_start(projq, projq_in.ap())
            watt_sb = const.tile([P, AT], BF)
            nc.scalar.dma_start(watt_sb, w_att.ap())

            # ---- epilogue pieces -------------------------------------
            def emit_scores(pts, sps=None):
                """Col-tiled scores: ONE PSUM bank, 4 stripes at partitions
                {0,32,64,96} via tile_position, start=False onto a zeroed
                bank; ONE whole-bank exp + esum (ACT's strict FIFO is the
                scarce resource in the final window - per-stripe exps
                serialize behind everything and block the PE queue)."""
                if sps is None:
                    sps = sc_ps.tile([P, SF], F32, name="sps")
                    nc.vector.memset(sps, 0.0)
                for at in range(AT):
                    for sc in range(SC):
                        nc.tensor.matmul(
                            sps[32 * sc : 32 * sc + 1, :],
                            watt_sb[:, at : at + 1],
                            pts[at][:, sc * SF : (sc + 1) * SF],
                            start=False,
                            stop=(at == AT - 1),
                            skip_group_check=True,
                            tile_position=(0, 32 * sc),
                        )
                exp_sb = small.tile([P, SF], BF, name="exp_sb")
                esum = small.tile([P, 1], F32, name="esum")
                nc.scalar.activation(exp_sb, sps, EXP, accum_out=esum)
                return exp_sb, esum

            def emit_wbc_dma(exp_sb):
                # gather the 4 stripe rows into DRAM, broadcast back to
                # all 128 partitions (stride-0 read); scalar ring
                exp_dram = dram.tile([1, S], BF, name="exp_dram")
                nc.scalar.dma_start(
                    bass.AP(
                        tensor=exp_dram.tensor,
                        offset=exp_dram.offset,
                        ap=[[SF, SC], [1, SF]],
                    ),
                    exp_sb[0 : 32 * SC - 31 : 32, :],
                )
                wbc = ttp.tile([P, S], BF, name="wbc")
                nc.scalar.dma_start(
                    wbc,
                    bass.AP(
                        tensor=exp_dram.tensor,
                        offset=exp_dram.offset,
                        ap=[[0, P], [1, S]],
                    ),
                )
                return wbc

            def emit_rcp(esum):
                """softmax denominator: tot = mask . esum (stripe rows),
                broadcast with a K=1 ones matmul, reciprocal on [128,1].
                Tiny PE matmuls; call only at points where ACT's exp has
                long completed (the in-order PE queue would stall)."""
                tot_ps = mm_ps.tile([P, SF], F32, name="mm_acc")
                nc.tensor.matmul(tot_ps[:1, :1], mask_f, esum, start=True, stop=True)
                tot_sb = small.tile([1, 1], F32, name="tot_sb")
                nc.scalar.copy(tot_sb, tot_ps[:1, :1])
                totbc_ps = mm_ps.tile([P, SF], F32, name="mm_acc")
                nc.tensor.matmul(totbc_ps[:, :1], ones_f, tot_sb, start=True, stop=True)
                totbc = small.tile([P, 1], F32, name="totbc")
                nc.vector.tensor_copy(totbc, totbc_ps[:, :1])
                rcp = small.tile([P, 1], F32, name="rcp")
                nc.vector.reciprocal(rcp, totbc)
                return rcp

            def ctx_mult(proj, wbc, scr, i):
                nc.vector.tensor_tensor(
                    out=scr, in0=proj[:, i * S : (i + 1) * S], in1=wbc, op=MULT
                )

            def ctx_reduce(scr, rcp, c, i, fold=False):
                if fold:
                    nc.vector.tensor_tensor(
                        out=scr[:, : S // 2], in0=scr[:, : S // 2],
                        in1=scr[:, S // 2 :], op=ADD,
                    )
                    nc.scalar.activation(
                        scr[:, : S // 2], scr[:, : S // 2], COPY, scale=rcp,
                        accum_out=c[:, i : i + 1],
                    )
                else:
                    nc.scalar.activation(
                        scr, scr, COPY, scale=rcp, accum_out=c[:, i : i + 1]
                    )

            def emit_store(c, bidx, single=False):
                nc.scalar.dma_start(
                    bass.AP(tensor=out, offset=bidx * A, ap=[[1, P], [P, AT]]),
                    c,
                    single_packet=single,
                )

            # ---- main batch loop -------------------------------------
            prev = None  # (batch_idx, t tiles, projTall)
            ep = {}  # in-flight deferred epilogue state
            for b in range(BPC):
                last = b == BPC - 1
                xquads = []
                for h in range(NQ):
                    xq = xtp.tile([P, 4, S], BF, name=f"xq{h}")
                    nparts = 2 if b == 0 else 1
                    step = 4 // nparts
                    for j in range(nparts):
                        nc.sync.dma_start(
                            xq[:, j * step : (j + 1) * step, :],
                            bass.AP(
                                tensor=xT,
                                offset=((b * NQ + h) * P) * (4 * S) + j * step * S,
                                ap=[[4 * S, P], [1, step * S]],
                            ),
                        )
                    xquads.append(xq)

                ts_ = []
                projTall = ttp.tile([P, AT * S], BF, name="projTall", bufs=3)
                for at in range(AT):
                    t_sb = ttp.tile([P, S], BF, name=f"t{at}")
                    pss = [mm_ps.tile([P, SF], F32, name="mm_acc") for _ in range(SC)]
                    sl_at = slice(at * S, (at + 1) * S)
                    if last and at == AT - 1:
                        # sc-OUTER: each s-chunk closes early; tanh reads
                        # its PSUM bank (bias fused) per chunk; b2's
                        # interleaved reduces fill the ACT slack
                        for sc in range(SC):
                            for ec in range(EC):
                                nc.tensor.matmul(
                                    pss[sc],
                                    w_sb[:, at, ec, :],
                                    xquads[ec // 4][
                                        :, ec % 4, sc * SF : (sc + 1) * SF
                                    ],
                                    start=(ec == 0),
                                    stop=(ec == EC - 1),
                                )
                            sl = slice(at * S + sc * SF, at * S + (sc + 1) * SF)
                            nc.scalar.activation(
                                t_sb[:, sc * SF : (sc + 1) * SF],
                                pss[sc],
                                TANH,
                                bias=projq[:, at, b : b + 1],
                            )
                            nc.vector.tensor_copy(projTall[:, sl], pss[sc])
                            if prev is not None and sc < 2:
                                # b2's late multiplies on DVE (scratch
                                # freed by the at2-boundary reduces)
                                ctx_mult(
                                    ep["proj"], ep["wbc"], ep["cscr"][sc], sc + 2
                                )
                    else:
                        for ec in range(EC):
                            for sc in range(SC):
                                nc.tensor.matmul(
                                    pss[sc],
                                    w_sb[:, at, ec, :],
                                    xquads[ec // 4][
                                        :, ec % 4, sc * SF : (sc + 1) * SF
                                    ],
                                    start=(ec == 0),
                                    stop=(ec == EC - 1),
                                )
                        # single PSUM reader (DVE cast) gates PSUM release;
                        # tanh reads the SBUF copy in ONE whole-row ACT op
                        for sc in range(SC):
                            sl = slice(at * S + sc * SF, at * S + (sc + 1) * SF)
                            nc.vector.tensor_copy(projTall[:, sl], pss[sc])
                        nc.scalar.activation(
                            t_sb,
                            projTall[:, sl_at],
                            TANH,
                            bias=projq[:, at, b : b + 1],
                        )
                    ts_.append(t_sb)

                    if at == 1 and prev is not None:
                        # deferred epilogue part 1: scores / exp / broadcast
                        ep["b"], ep["ts"], ep["proj"] = prev
                        ep["exp"], ep["esum"] = emit_scores(ep["ts"])
                        ep["wbc"] = emit_wbc_dma(ep["exp"])
                        ep["cscr"] = [
                            ttp.tile([P, S], BF, name=f"cscr{i}", bufs=1)
                            for i in range(2)
                        ]
                        if last:
                            # pre-allocate + memset the FINAL scores bank
                            # (its memset must not queue behind epilogue
                            # DVE work and idle the PE)
                            ep["fsps"] = sc_ps.tile([P, SF], F32, name="sps")
                            nc.vector.memset(ep["fsps"], 0.0)

                    if at == 2 and last and prev is not None:
                        # b2 epilogue spread: denominator + first two
                        # multiply/reduce pairs in the at2 window (ACT has
                        # slack here; the final window does not)
                        ep["rcp"] = emit_rcp(ep["esum"])
                        ep["c"] = small.tile([P, AT], F32, name="c")
                        for i in range(2):
                            ctx_mult(ep["proj"], ep["wbc"], ep["cscr"][i], i)
                            ctx_reduce(
                                ep["cscr"][i], ep["rcp"], ep["c"], i, fold=True
                            )

                # non-last carriers: whole deferred ctx after the main MMs
                if prev is not None and not last:
                    ep["rcp"] = emit_rcp(ep["esum"])
                    ep["c"] = small.tile([P, AT], F32, name="c")
                    for i in range(AT):
                        scr = ep["cscr"][i % 2]
                        ctx_mult(ep["proj"], ep["wbc"], scr, i)
                        ctx_reduce(scr, ep["rcp"], ep["c"], i)
                    emit_store(ep["c"], ep["b"])

                prev = (b, ts_, projTall)

            # ---- final epilogue (latency-critical, PE idle afterwards) --
            pb, pts, pproj = prev
            exp_sb, esum = emit_scores(pts, sps=ep["fsps"])
            # b2's last two reduces + store fill ACT's exp->broadcast
            # latency window
            ctx_reduce(ep["cscr"][0], ep["rcp"], ep["c"], 2, fold=True)
            ctx_reduce(ep["cscr"][1], ep["rcp"], ep["c"], 3, fold=True)
            emit_store(ep["c"], ep["b"])
            # PE K=1 ones-matmul broadcast of the exp stripe rows
            wbc = ttp.tile([P, S], BF, name="wbc")
            wpss = []
            for sc in range(SC):
                wps = mm_ps.tile([P, SF], F32, name="mm_acc")
                nc.tensor.matmul(
                    wps,
                    ones2[32 * sc : 32 * sc + 1, :],
                    exp_sb[32 * sc : 32 * sc + 1, :],
                    start=True,
                    stop=True,
                    tile_position=(32 * sc, 0),
                )
                wpss.append(wps)
            rcp = emit_rcp(esum)
            for sc in range(SC):
                dst = wbc[:, sc * SF : (sc + 1) * SF]
                if sc % 2 == 0:
                    nc.vector.tensor_copy(dst, wpss[sc])
                else:
                    nc.scalar.copy(dst, wpss[sc])

            # final ctx on 4 scratch tiles (no WAR serialization); first
            # multiply chunked so it starts on the first broadcast chunk;
            # all reduces fold-halved to balance ACT vs DVE
            cscrs = [ttp.tile([P, S], BF, name=f"fscr{i}", bufs=1) for i in range(AT)]
            c = small.tile([P, AT], F32, name="c")
            for sc in range(SC):
                sl = slice(sc * SF, (sc + 1) * SF)
                nc.vector.tensor_tensor(
                    out=cscrs[0][:, sl], in0=pproj[:, sl], in1=wbc[:, sl], op=MULT
                )
            ctx_reduce(cscrs[0], rcp, c, 0, fold=True)
            for i in (1, 2, 3):
                ctx_mult(pproj, wbc, cscrs[i], i)
                ctx_reduce(cscrs[i], rcp, c, i, fold=True)
            # sync ring: it has been idle for ~50us (its completion-poke
            # path is drained), unlike the scalar ring
            nc.sync.dma_start(
                bass.AP(tensor=out, offset=pb * A, ap=[[1, P], [P, AT]]),
                c,
            )

    nc.compile()
    return nc


_nc = None


def prep_in_maps(inputs, query, W_in, W_q, w_att):
    """Host-side shard + pre-layout: every DRAM operand is arranged so
    each partition's line is one long contiguous DMA descriptor."""
    bf = ml_dtypes.bfloat16
    x_bf = np.asarray(inputs).astype(bf)
    # [b, s, e] -> [b, e, s] -> chunks [b, c, p, s] -> [b, h, p, j, s]
    xT_bf = x_bf.transpose(0, 2, 1).reshape(B, NQ, 4, P, S).transpose(0, 1, 3, 2, 4)
    xT_bf = np.ascontiguousarray(xT_bf).reshape(B, NQ, P, 4 * S)
    # w_in[e, a] -> [p, at, ec, j] with e = ec*128+p, a = at*128+j
    w_in_pre = np.ascontiguousarray(
        np.asarray(W_in).astype(bf).reshape(EC, P, AT, P).transpose(1, 2, 0, 3)
    )
    # w_att[a] -> [p, at] with a = at*128+p
    w_att_pre = np.ascontiguousarray(
        np.asarray(w_att).astype(bf).reshape(AT, P).T
    )
    # proj_q on host, in bf16-rounded operands to match device numerics
    pq = (
        np.asarray(query).astype(bf).astype(np.float32)
        @ np.asarray(W_q).astype(bf).astype(np.float32)
    )  # [B, A] f32

    in_maps = []
    for c in range(NCORES):
        sl = slice(c * BPC, (c + 1) * BPC)
        # proj_q[b, a] -> [p, at, b] with a = at*128+p
        pq_pre = np.ascontiguousarray(
            pq[sl].reshape(BPC, AT, P).transpose(2, 1, 0).astype(np.float32)
        )
        in_maps.append(
            {
                "xT": xT_bf[sl],
                "w_in": w_in_pre,
                "w_att": w_att_pre,
                "projq": pq_pre,
            }
        )
    return in_maps


def kernel(inputs, query, W_in, W_q, w_att):
    global _nc
    if _nc is None:
        _nc = build()
    in_maps = prep_in_maps(inputs, query, W_in, W_q, w_att)
    res = bass_utils.run_bass_kernel_spmd(_nc, in_maps, core_ids=list(range(NCORES)))
    return np.concatenate([r["out"] for r in res.results], axis=0)


if __name__ == "__main__":
    rng = np.random.default_rng(0)
    ins = {
        "inputs": rng.standard_normal((B, S, E), dtype=np.float32),
        "query": rng.standard_normal((B, Q), dtype=np.float32),
        "W_in": (rng.standard_normal((E, A), dtype=np.float32) / np.sqrt(E)).astype(
            np.float32
        ),
        "W_q": (rng.standard_normal((Q, A), dtype=np.float32) / np.sqrt(Q)).astype(
            np.float32
        ),
        "w_att": (rng.standard_normal((A,), dtype=np.float32) / np.sqrt(A)).astype(
            np.float32
        ),
    }
    got = kernel(**ins)
    print("out shape", got.shape, got.dtype)


# revision 36
# speedup vs baseline: 1.0822x; 1.0277x over previous
"""Bahdanau (additive) attention TRN2 Bass kernel (v10).

reference:
    proj_in = einsum("bse,ea->bsa", inputs, W_in)      # [B,S,A]
    proj_q  = (query @ W_q)[:, None, :]                # [B,1,A]
    scores  = einsum("bsa,a->bs", tanh(proj_in+proj_q), w_att)
    weights = softmax(scores, axis=1)
    context = einsum("bs,bsa->ba", weights, proj_in)   # [B,A]

B,S,E,Q,A = 32,2048,1024,1024,512.

Sharding: data-parallel over batch. 8 cores x 4 batches each; weights
replicated. No collectives; host scatters inputs / gathers outputs.

Measured mechanics this kernel is built around (v6..v9c traces):
  - DMA descriptor generation is ~260ns/descriptor/queue at the head:
    all DRAM operands are host-laid so every partition line is one
    long contiguous descriptor (x: 16KB quads / 8KB b0-pairs; w_in:
    2KB at-slices; projq 64B; watt 8B).
  - Every hardware-DGE DMA's completion semaphore lands ~2us (early)
    to ~10us (late kernel) after its data; the end-of-kernel barrier
    waits on ALL of them, so the final store eats the full lag. The
    last x/wbc/store DMAs are therefore issued as early as possible.
  - The in-order PE queue must never hold an instruction whose wait
    depends on ACT/DVE progress while main MMs remain (denominator
    matmuls go after the carrier's main MMs; the final scores bank is
    memset early).
  - ACT op overhead ~350cyc -> tanh batched [P,S] per at; ACT reduces
    use the per-partition scale operand to fold the softmax normalize.
  - HAM: ~3.4us of sustained PE activity re-clocks 1.2->2.4GHz; DMA-
    independent dummy MMs warm it during the load ramp.
  - proj_q is computed on the host (trivial 32x512 matmul).

Structure:
  - per batch: 2 quad x-DMAs (b0: 4 pair-DMAs); at-groups ec-outer
    sc-inner accumulating 4 PSUM banks; DVE casts PSUM->projTall bf16
    (sole PSUM reader); one batched tanh per at (bias fused).
  - deferred epilogue for batch b runs in carrier b+1: scores via
    col-tiled stripes into ONE PSUM bank (tile_position), whole-bank
    exp + esum, DRAM-bounce broadcast of the weights, PE mask-dot
    denominator, DVE multiply + ACT scale-reduce ctx, scalar-ring
    store.
  - b2's epilogue interleaves into b3's at2/at3 windows (fold-halved
    reduces) so the final epilogue's exp isn't queued behind it.
  - final epilogue: sc-outer last at-group, per-sc scores/exp/PE-
    broadcast pipeline, ctx on 4 scratch tiles, single-packet store.
"""

import sys

sys.path.insert(0, "/opt/trn_rl_repo")

import ml_dtypes
import numpy as np

import concourse.bass as bass
import concourse.tile as tile
from concourse import bacc, bass_utils, mybir

B, S, E, Q, A = 32, 2048, 1024, 1024, 512
NCORES = 8
BPC = B // NCORES  # batches per core
P = 128
EC = E // P  # 8 e-chunks
AT = A // P  # 4 a-tiles
SF = 512  # matmul moving free dim
SC = S // SF  # 4 s-chunks
NQ = EC // 4  # quad tiles per batch (2)

BF = mybir.dt.bfloat16
F32 = mybir.dt.float32
TANH = mybir.ActivationFunctionType.Tanh
EXP = mybir.ActivationFunctionType.Exp
COPY = mybir.ActivationFunctionType.Copy
MULT = mybir.AluOpType.mult
ADD = mybir.AluOpType.add


def build():
    nc = bacc.Bacc("TRN2", target_bir_lowering=False, debug=False)

    xT = nc.dram_tensor("xT", [BPC, NQ, P, 4 * S], BF, kind="ExternalInput")
    # w_in host-laid [p, at, ec, j]: per-at slices are 2KB/partition
    w_in = nc.dram_tensor("w_in", [P, AT, EC, P], BF, kind="ExternalInput")
    # w_att host-laid [p, at]: 8B/partition lines
    w_att = nc.dram_tensor("w_att", [P, AT], BF, kind="ExternalInput")
    projq_in = nc.dram_tensor("projq", [P, AT, BPC], F32, kind="ExternalInput")
    out = nc.dram_tensor("out", [BPC, A], F32, kind="ExternalOutput")

    with tile.TileContext(nc) as tc:
        with (
            tc.tile_pool(name="const", bufs=1) as const,
            tc.tile_pool(name="xtp", bufs=2) as xtp,
            tc.tile_pool(name="ttp", bufs=2) as ttp,
            tc.tile_pool(name="small", bufs=3) as small,
            tc.tile_pool(name="mm_ps", bufs=7, space="PSUM") as mm_ps,
            tc.tile_pool(name="sc_ps", bufs=1, space="PSUM") as sc_ps,
            tc.tile_pool(name="dram", bufs=2, space="DRAM") as dram,
        ):
            # ---- tiny constants + HAM warmup (no DMA dependency) -----
            ones2 = const.tile([P, P], BF)
            nc.vector.memset(ones2, 1.0)
            dummy = const.tile([P, SF], BF)
            nc.vector.memset(dummy, 0.0)
            warm_ps = mm_ps.tile([P, SF], F32, name="mm_acc")
            NWARM = 14
            for i in range(NWARM):
                nc.tensor.matmul(
                    warm_ps, ones2, dummy, start=(i == 0), stop=(i == NWARM - 1)
                )
            # tiny reader so the warmup bank has a tracked release (DVE:
            # on the Scalar queue it delays the weight DMA issues)
            warm_scr = small.tile([1, 1], F32, name="warm_scr")
            nc.vector.tensor_copy(warm_scr, warm_ps[:1, :1])

            ones_f = const.tile([1, P], F32)
            nc.vector.memset(ones_f, 1.0)
            mask_f = const.tile([P, 1], F32)
            nc.vector.memset(mask_f, 0.0)
            for sc in range(SC):
                nc.vector.memset(mask_f[32 * sc : 32 * sc + 1, :], 1.0)

            # ---- weights on the scalar ring: at0's slice first so the
            # first MM group is gated on 256KB, not 1MB.
            w_sb = const.tile([P, AT, EC, P], BF)
            nc.scalar.dma_start(w_sb[:, :1], w_in.ap()[:, :1])
            nc.scalar.dma_start(w_sb[:, 1:], w_in.ap()[:, 1:])
            projq = const.tile([P, AT, BPC], F32)
            nc.scalar.dma_start(projq, projq_in.ap())
            watt_sb = const.tile([P, AT], BF)
            nc.scalar.dma_start(watt_sb, w_att.ap())

            # ---- epilogue pieces -------------------------------------
            def emit_scores(pts, sps=None):
                """Col-tiled scores: ONE PSUM bank, 4 stripes at partitions
                {0,32,64,96} via tile_position, start=False onto a zeroed
                bank; ONE whole-bank exp + esum (ACT's strict FIFO is the
                scarce resource in the final window - per-stripe exps
                serialize behind everything and block the PE queue)."""
                if sps is None:
                    sps = sc_ps.tile([P, SF], F32, name="sps")
                    nc.vector.memset(sps, 0.0)
                for at in range(AT):
                    for sc in range(SC):
                        nc.tensor.matmul(
                            sps[32 * sc : 32 * sc + 1, :],
                            watt_sb[:, at : at + 1],
                            pts[at][:, sc * SF : (sc + 1) * SF],
                            start=False,
                            stop=(at == AT - 1),
                            skip_group_check=True,
                            tile_position=(0, 32 * sc),
                        )
                exp_sb = small.tile([P, SF], BF, name="exp_sb")
                esum = small.tile([P, 1], F32, name="esum")
                nc.scalar.activation(exp_sb, sps, EXP, accum_out=esum)
                return exp_sb, esum

            def emit_wbc_dma(exp_sb):
                # gather the 4 stripe rows into DRAM, broadcast back to
                # all 128 partitions (stride-0 read); scalar ring
                exp_dram = dram.tile([1, S], BF, name="exp_dram")
                nc.scalar.dma_start(
                    bass.AP(
                        tensor=exp_dram.tensor,
                        offset=exp_dram.offset,
                        ap=[[SF, SC], [1, SF]],
                    ),
                    exp_sb[0 : 32 * SC - 31 : 32, :],
                )
                wbc = ttp.tile([P, S], BF, name="wbc")
                nc.scalar.dma_start(
                    wbc,
                    bass.AP(
                        tensor=exp_dram.tensor,
                        offset=exp_dram.offset,
                        ap=[[0, P], [1, S]],
                    ),
                )
                return wbc

            def emit_rcp(esum):
                """softmax denominator: tot = mask . esum (stripe rows),
                broadcast with a K=1 ones matmul, reciprocal on [128,1].
                Tiny PE matmuls; call only at points where ACT's exp has
                long completed (the in-order PE queue would stall)."""
                tot_ps = mm_ps.tile([P, SF], F32, name="mm_acc")
                nc.tensor.matmul(tot_ps[:1, :1], mask_f, esum, start=True, stop=True)
                tot_sb = small.tile([1, 1], F32, name="tot_sb")
                nc.scalar.copy(tot_sb, tot_ps[:1, :1])
                totbc_ps = mm_ps.tile([P, SF], F32, name="mm_acc")
                nc.tensor.matmul(totbc_ps[:, :1], ones_f, tot_sb, start=True, stop=True)
                totbc = small.tile([P, 1], F32, name="totbc")
                nc.vector.tensor_copy(totbc, totbc_ps[:, :1])
                rcp = small.tile([P, 1], F32, name="rcp")
                nc.vector.reciprocal(rcp, totbc)
                return rcp

            def ctx_mult(proj, wbc, scr, i):
                nc.vector.tensor_tensor(
                    out=scr, in0=proj[:, i * S : (i + 1) * S], in1=wbc, op=MULT
                )

            def ctx_reduce(scr, rcp, c, i, fold=False):
                if fold:
                    nc.vector.tensor_tensor(
                        out=scr[:, : S // 2], in0=scr[:, : S // 2],
                        in1=scr[:, S // 2 :], op=ADD,
                    )
                    nc.scalar.activation(
                        scr[:, : S // 2], scr[:, : S // 2], COPY, scale=rcp,
                        accum_out=c[:, i : i + 1],
                    )
                else:
                    nc.scalar.activation(
                        scr, scr, COPY, scale=rcp, accum_out=c[:, i : i + 1]
                    )

            def emit_store(c, bidx, single=False):
                nc.scalar.dma_start(
                    bass.AP(tensor=out, offset=bidx * A, ap=[[1, P], [P, AT]]),
                    c,
                    single_packet=single,
                )

            # ---- main batch loop -------------------------------------
            prev = None  # (batch_idx, t tiles, projTall)
            ep = {}  # in-flight deferred epilogue state
            for b in range(BPC):
                last = b == BPC - 1
                xquads = []
                for h in range(NQ):
                    xq = xtp.tile([P, 4, S], BF, name=f"xq{h}")
                    if b == 0:
                        # finest granularity first: the first MM group is
                        # gated on chunk 0's completion POKES (data +2 to
                        # +5us), so the first DMA is a single 0.5MB chunk
                        parts = [(0, 1), (1, 1), (2, 2)] if h == 0 else [(0, 2), (2, 2)]
                    else:
                        parts = [(0, 4)]
                    for j0, step in parts:
                        nc.sync.dma_start(
                            xq[:, j0 : j0 + step, :],
                            bass.AP(
                                tensor=xT,
                                offset=((b * NQ + h) * P) * (4 * S) + j0 * S,
                                ap=[[4 * S, P], [1, step * S]],
                            ),
                        )
                    xquads.append(xq)

                ts_ = []
                projTall = ttp.tile([P, AT * S], BF, name="projTall", bufs=3)
                for at in range(AT):
                    t_sb = ttp.tile([P, S], BF, name=f"t{at}")
                    pss = [mm_ps.tile([P, SF], F32, name="mm_acc") for _ in range(SC)]
                    sl_at = slice(at * S, (at + 1) * S)
                    if last and at == AT - 1:
                        # sc-OUTER: each s-chunk closes early; tanh reads
                        # its PSUM bank (bias fused) per chunk; b2's
                        # interleaved reduces fill the ACT slack
                        for sc in range(SC):
                            for ec in range(EC):
                                nc.tensor.matmul(
                                    pss[sc],
                                    w_sb[:, at, ec, :],
                                    xquads[ec // 4][
                                        :, ec % 4, sc * SF : (sc + 1) * SF
                                    ],
                                    start=(ec == 0),
                                    stop=(ec == EC - 1),
                                )
                            sl = slice(at * S + sc * SF, at * S + (sc + 1) * SF)
                            nc.scalar.activation(
                                t_sb[:, sc * SF : (sc + 1) * SF],
                                pss[sc],
                                TANH,
                                bias=projq[:, at, b : b + 1],
                            )
                            nc.vector.tensor_copy(projTall[:, sl], pss[sc])
                            if prev is not None and sc < 2:
                                # b2's late multiplies on DVE (scratch
                                # freed by the at2-boundary reduces)
                                ctx_mult(
                                    ep["proj"], ep["wbc"], ep["cscr"][sc], sc + 2
                                )
                            if prev is not None and sc >= 2:
                                # b2's last reduces slot into the at3
                                # window's ACT slack; in the final window
                                # the dependency-driven scheduler would
                                # run them BEFORE the exp (their inputs
                                # ready earlier) and delay the broadcast
                                ctx_reduce(
                                    ep["cscr"][sc - 2], ep["rcp"], ep["c"],
                                    sc, fold=True,
                                )
                        if prev is not None:
                            emit_store(ep["c"], ep["b"])
                    else:
                        for ec in range(EC):
                            for sc in range(SC):
                                nc.tensor.matmul(
                                    pss[sc],
                                    w_sb[:, at, ec, :],
                                    xquads[ec // 4][
                                        :, ec % 4, sc * SF : (sc + 1) * SF
                                    ],
                                    start=(ec == 0),
                                    stop=(ec == EC - 1),
                                )
                        # single PSUM reader (DVE cast) gates PSUM release;
                        # tanh reads the SBUF copy in ONE whole-row ACT op
                        for sc in range(SC):
                            sl = slice(at * S + sc * SF, at * S + (sc + 1) * SF)
                            nc.vector.tensor_copy(projTall[:, sl], pss[sc])
                        nc.scalar.activation(
                            t_sb,
                            projTall[:, sl_at],
                            TANH,
                            bias=projq[:, at, b : b + 1],
                        )
                    ts_.append(t_sb)

                    if at == 1 and prev is not None:
                        # deferred epilogue part 1: scores / exp / broadcast
                        ep["b"], ep["ts"], ep["proj"] = prev
                        ep["exp"], ep["esum"] = emit_scores(ep["ts"])
                        ep["wbc"] = emit_wbc_dma(ep["exp"])
                        ep["cscr"] = [
                            ttp.tile([P, S], BF, name=f"cscr{i}", bufs=1)
                            for i in range(2)
                        ]
                        if last:
                            # pre-allocate + memset the FINAL scores bank
                            # (its memset must not queue behind epilogue
                            # DVE work and idle the PE)
                            ep["fsps"] = sc_ps.tile([P, SF], F32, name="sps")
                            nc.vector.memset(ep["fsps"], 0.0)

                    if at == 2 and last and prev is not None:
                        # b2 epilogue spread: denominator + first two
                        # multiply/reduce pairs in the at2 window (ACT has
                        # slack here; the final window does not)
                        ep["rcp"] = emit_rcp(ep["esum"])
                        ep["c"] = small.tile([P, AT], F32, name="c")
                        for i in range(2):
                            ctx_mult(ep["proj"], ep["wbc"], ep["cscr"][i], i)
                            ctx_reduce(
                                ep["cscr"][i], ep["rcp"], ep["c"], i, fold=True
                            )

                # non-last carriers: whole deferred ctx after the main MMs
                if prev is not None and not last:
                    ep["rcp"] = emit_rcp(ep["esum"])
                    ep["c"] = small.tile([P, AT], F32, name="c")
                    for i in range(AT):
                        scr = ep["cscr"][i % 2]
                        ctx_mult(ep["proj"], ep["wbc"], scr, i)
                        ctx_reduce(scr, ep["rcp"], ep["c"], i)
                    emit_store(ep["c"], ep["b"])

                prev = (b, ts_, projTall)

            # ---- final epilogue (latency-critical, PE idle afterwards) --
            pb, pts, pproj = prev
            exp_sb, esum = emit_scores(pts, sps=ep["fsps"])
            # PE K=1 ones-matmul broadcast of the exp stripe rows
            wbc = ttp.tile([P, S], BF, name="wbc")
            wpss = []
            for sc in range(SC):
                wps = mm_ps.tile([P, SF], F32, name="mm_acc")
                nc.tensor.matmul(
                    wps,
                    ones2[32 * sc : 32 * sc + 1, :],
                    exp_sb[32 * sc : 32 * sc + 1, :],
                    start=True,
                    stop=True,
                    tile_position=(32 * sc, 0),
                )
                wpss.append(wps)
            rcp = emit_rcp(esum)
            for sc in range(SC):
                dst = wbc[:, sc * SF : (sc + 1) * SF]
                if sc % 2 == 0:
                    nc.vector.tensor_copy(dst, wpss[sc])
                else:
                    nc.scalar.copy(dst, wpss[sc])

            # final ctx: fused DVE multiply+scale+reduce per at (the only
            # fused form codegen accepts); at0 chunked over s so it
            # starts on the first broadcast chunk. ACT stays free for the
            # exp - the chain is DVE-throughput-bound only.
            cscrs = [ttp.tile([P, S], BF, name=f"fscr{i}", bufs=1) for i in range(2)]
            c = small.tile([P, AT], F32, name="c")
            part = small.tile([P, SC], F32, name="part")
            for sc in range(SC):
                sl = slice(sc * SF, (sc + 1) * SF)
                nc.vector.scalar_tensor_tensor(
                    out=cscrs[0][:, sl], in0=pproj[:, sl], scalar=rcp,
                    in1=wbc[:, sl], op0=MULT, op1=MULT,
                    accum_out=part[:, sc : sc + 1],
                )
            for i in (1, 2, 3):
                nc.vector.scalar_tensor_tensor(
                    out=cscrs[i % 2], in0=pproj[:, i * S : (i + 1) * S], scalar=rcp,
                    in1=wbc, op0=MULT, op1=MULT,
                    accum_out=c[:, i : i + 1],
                )
            # fold at0's 4 chunk-partials on ACT (idle here)
            pscr = small.tile([P, SC], F32, name="pscr")
            nc.scalar.activation(pscr, part, COPY, accum_out=c[:, 0:1])
            # sync ring: it has been idle for ~50us (its completion-poke
            # path is drained), unlike the scalar ring
            nc.sync.dma_start(
                bass.AP(tensor=out, offset=pb * A, ap=[[1, P], [P, AT]]),
                c,
            )

    nc.compile()
    return nc


_nc = None


def prep_in_maps(inputs, query, W_in, W_q, w_att):
    """Host-side shard + pre-layout: every DRAM operand is arranged so
    each partition's line is one long contiguous DMA descriptor."""
    bf = ml_dtypes.bfloat16
    x_bf = np.asarray(inputs).astype(bf)
    # [b, s, e] -> [b, e, s] -> chunks [b, c, p, s] -> [b, h, p, j, s]
    xT_bf = x_bf.transpose(0, 2, 1).reshape(B, NQ, 4, P, S).transpose(0, 1, 3, 2, 4)
    xT_bf = np.ascontiguousarray(xT_bf).reshape(B, NQ, P, 4 * S)
    # w_in[e, a] -> [p, at, ec, j] with e = ec*128+p, a = at*128+j
    w_in_pre = np.ascontiguousarray(
        np.asarray(W_in).astype(bf).reshape(EC, P, AT, P).transpose(1, 2, 0, 3)
    )
    # w_att[a] -> [p, at] with a = at*128+p
    w_att_pre = np.ascontiguousarray(
        np.asarray(w_att).astype(bf).reshape(AT, P).T
    )
    # proj_q on host, in bf16-rounded operands to match device numerics
    pq = (
        np.asarray(query).astype(bf).astype(np.float32)
        @ np.asarray(W_q).astype(bf).astype(np.float32)
    )  # [B, A] f32

    in_maps = []
    for c in range(NCORES):
        sl = slice(c * BPC, (c + 1) * BPC)
        # proj_q[b, a] -> [p, at, b] with a = at*128+p
        pq_pre = np.ascontiguousarray(
            pq[sl].reshape(BPC, AT, P).transpose(2, 1, 0).astype(np.float32)
        )
        in_maps.append(
            {
                "xT": xT_bf[sl],
                "w_in": w_in_pre,
                "w_att": w_att_pre,
                "projq": pq_pre,
            }
        )
    return in_maps


def kernel(inputs, query, W_in, W_q, w_att):
    global _nc
    if _nc is None:
        _nc = build()
    in_maps = prep_in_maps(inputs, query, W_in, W_q, w_att)
    res = bass_utils.run_bass_kernel_spmd(_nc, in_maps, core_ids=list(range(NCORES)))
    return np.concatenate([r["out"] for r in res.results], axis=0)


if __name__ == "__main__":
    rng = np.random.default_rng(0)
    ins = {
        "inputs": rng.standard_normal((B, S, E), dtype=np.float32),
        "query": rng.standard_normal((B, Q), dtype=np.float32),
        "W_in": (rng.standard_normal((E, A), dtype=np.float32) / np.sqrt(E)).astype(
            np.float32
        ),
        "W_q": (rng.standard_normal((Q, A), dtype=np.float32) / np.sqrt(Q)).astype(
            np.float32
        ),
        "w_att": (rng.standard_normal((A,), dtype=np.float32) / np.sqrt(A)).astype(
            np.float32
        ),
    }
    got = kernel(**ins)
    print("out shape", got.shape, got.dtype)
